# revision 1
# baseline (speedup 1.0000x reference)
"""MoE (8 routed experts, top-2, + shared expert) on 8 TRN2 NeuronCores.

Strategy: expert-parallel. Host computes the gate (fp32 numpy, exactly
mirroring the reference), gathers each expert's tokens, and core e runs
expert e's SwiGLU (h = silu(x@w1T) * (x@w3T) * cw; y = h_bf16 @ w2T)
over its gathered tokens, plus a 1/8 token-slice of the shared expert.
Host scatters expert outputs back and combines in bf16 expert order.

All tensors fed to the device are pre-arranged on host into
partition-major layouts so every DMA is contiguous per partition:
  activations/weights for matmul lhsT/rhs always have the contraction
  dim chunked as [pi=128, po, free].
"""

import numpy as np
import ml_dtypes

import concourse.mybir as mybir
from concourse import bacc
from concourse.tile import TileContext
from concourse import bass_utils

BF16 = mybir.dt.bfloat16
F32 = mybir.dt.float32

D = 2048          # model dim
I = 1408          # expert inter dim
E = 8             # routed experts
TOPK = 2
N_CORES = 8
DPO = D // 128    # 16 chunks of the model dim
IPO = I // 128    # 11 chunks of the inter dim

_BUILD_CACHE = {}


def _c_blocks(C):
    """Split C columns into equal-ish blocks <= 512, multiples of 128."""
    nb = -(-C // 512)
    per = -(-C // (nb * 128)) * 128
    blocks = []
    off = 0
    while off < C:
        w = min(per, C - off)
        blocks.append((off, w))
        off += w
    return blocks


def _build(C, TS):
    """Build the per-core Bass kernel for routed capacity C and shared
    token-slice TS. Same NEFF runs SPMD on all 8 cores."""
    nc = bacc.Bacc("TRN2", debug=False, enable_asserts=False,
                   num_devices=N_CORES, enable_partition_id=False)

    def din(name, shape, dt=BF16):
        return nc.dram_tensor(name, shape, dt, kind="ExternalInput").ap()

    def dout(name, shape, dt=BF16):
        return nc.dram_tensor(name, shape, dt, kind="ExternalOutput").ap()

    xr = din("xr", [128, DPO, C])            # routed tokens, [d_pi, d_po, c]
    xs = din("xs", [128, DPO, TS])           # shared-expert token slice
    cwb = din("cwb", [128, C], F32)          # combine weights, replicated
    w1t = din("w1t", [IPO, 128, D])          # [i_blk][d_pi][d_po*128+i_c]
    w3t = din("w3t", [IPO, 128, D])
    w2t = din("w2t", [DPO, 128, I])          # [d_blk][i_pi][i_po*128+d_c]
    sw1t = din("sw1t", [IPO, 128, D])
    sw3t = din("sw3t", [IPO, 128, D])
    sw2t = din("sw2t", [DPO, 128, I])
    ye = dout("ye", [128, DPO, C])           # [d_pi, d_po, c]
    zs = dout("zs", [128, DPO, TS])

    Silu = mybir.ActivationFunctionType.Silu

    with TileContext(nc) as tc:
        with tc.tile_pool(name="main", bufs=1) as pool, \
             tc.tile_pool(name="psum", bufs=1, space="PSUM") as pp:
            cw_sb = pool.tile([128, C], F32, tag="cwb", bufs=1, name="cw_sb")

            # routed job first: the second job's startup stream then
            # overlaps the first job's ~113us of phase-B PE work, and the
            # small xs stream doesn't starve routed phase-B weight loads
            jobs = [
                ("r", C, xr, w1t, w3t, w2t, ye, True),
                ("s", TS, xs, sw1t, sw3t, sw2t, zs, False),
            ]
            for jname, CJ, x_d, w1_d, w3_d, w2_d, out_d, scaled in jobs:
                cbs = _c_blocks(CJ)
                x_sb = pool.tile([128, DPO, CJ], BF16, tag=f"x_{jname}",
                                 bufs=1, name=f"x_{jname}")
                # startup: land just enough bytes for the first matmuls
                # (x slice 0 + the first weight chunks) before streaming
                # the rest, so the PE starts ~10us in instead of ~25us
                nc.sync.dma_start(x_sb[:, 0, :], x_d[:, 0, :])
                w13_first = []
                wdr = []
                for wd, wn in ((w1_d, "w1"), (w3_d, "w3")):
                    w_sb = pool.tile([128, DPO, 128], BF16, tag="w13",
                                     bufs=6, name=f"{wn}_{jname}_0")
                    w13_first.append(w_sb)
                    wdr.append(wd[0].rearrange("p (a b) -> p a b", a=DPO))
                for w_sb, wsrc in zip(w13_first, wdr):
                    nc.sync.dma_start(w_sb[:, 0:4, :], wsrc[:, 0:4, :])
                for w_sb, wsrc in zip(w13_first, wdr):
                    nc.sync.dma_start(w_sb[:, 4:, :], wsrc[:, 4:, :])
                for dsl in range(1, DPO):
                    nc.sync.dma_start(x_sb[:, dsl, :], x_d[:, dsl, :])
                if scaled:
                    nc.sync.dma_start(cw_sb[:], cwb[:])
                H = pool.tile([128, IPO, CJ], BF16, tag=f"H_{jname}",
                              bufs=1, name=f"H_{jname}")

                # ---- phase A: H = silu(x@w1T) * (x@w3T) [* cw] ----
                for i in range(IPO):
                    if i == 0:
                        w1_sb, w3_sb = w13_first
                    else:
                        w1_sb = pool.tile([128, DPO, 128], BF16, tag="w13",
                                          bufs=6, name=f"w1_{jname}_{i}")
                        nc.sync.dma_start(
                            w1_sb[:],
                            w1_d[i].rearrange("p (a b) -> p a b", a=DPO))
                        w3_sb = pool.tile([128, DPO, 128], BF16, tag="w13",
                                          bufs=6, name=f"w3_{jname}_{i}")
                        nc.sync.dma_start(
                            w3_sb[:],
                            w3_d[i].rearrange("p (a b) -> p a b", a=DPO))
                    p1s = []
                    p3s = []
                    for bi, (off, w) in enumerate(cbs):
                        p1s.append(pp.tile([128, w], F32, tag="ps", bufs=8,
                                           name=f"p1_{jname}_{i}_{bi}"))
                        p3s.append(pp.tile([128, w], F32, tag="ps", bufs=8,
                                           name=f"p3_{jname}_{i}_{bi}"))
                    for d in range(DPO):
                        for bi, (off, w) in enumerate(cbs):
                            nc.tensor.matmul(
                                p1s[bi][:], w1_sb[:, d, :],
                                x_sb[:, d, off:off + w],
                                start=(d == 0), stop=(d == DPO - 1))
                        for bi, (off, w) in enumerate(cbs):
                            nc.tensor.matmul(
                                p3s[bi][:], w3_sb[:, d, :],
                                x_sb[:, d, off:off + w],
                                start=(d == 0), stop=(d == DPO - 1))
                    for bi, (off, w) in enumerate(cbs):
                        s_t = pool.tile([128, w], F32, tag="act1", bufs=6,
                                        name=f"s_{jname}_{i}_{bi}")
                        nc.scalar.activation(s_t[:], p1s[bi][:], Silu)
                        if scaled:
                            t_t = pool.tile([128, w], F32, tag="act2", bufs=6,
                                            name=f"t_{jname}_{i}_{bi}")
                            nc.vector.tensor_mul(t_t[:], p3s[bi][:],
                                                 cw_sb[:, off:off + w])
                            nc.vector.tensor_mul(H[:, i, off:off + w],
                                                 s_t[:], t_t[:])
                        else:
                            nc.vector.tensor_mul(H[:, i, off:off + w],
                                                 s_t[:], p3s[bi][:])

                # ---- phase B: out = H @ w2T ----
                for do in range(DPO):
                    w2_sb = pool.tile([128, IPO, 128], BF16, tag="w2",
                                      bufs=5, name=f"w2_{jname}_{do}")
                    nc.sync.dma_start(
                        w2_sb[:], w2_d[do].rearrange("p (a b) -> p a b", a=IPO))
                    pys = []
                    for bi, (off, w) in enumerate(cbs):
                        pys.append(pp.tile([128, w], F32, tag="ps", bufs=8,
                                           name=f"py_{jname}_{do}_{bi}"))
                    for i in range(IPO):
                        for bi, (off, w) in enumerate(cbs):
                            nc.tensor.matmul(
                                pys[bi][:], w2_sb[:, i, :],
                                H[:, i, off:off + w],
                                start=(i == 0), stop=(i == IPO - 1))
                    for bi, (off, w) in enumerate(cbs):
                        y_t = pool.tile([128, w], BF16, tag="yo", bufs=8,
                                        name=f"y_{jname}_{do}_{bi}")
                        nc.vector.tensor_copy(y_t[:], pys[bi][:])
                        nc.sync.dma_start(out_d[:, do, off:off + w], y_t[:])

    nc.finalize()
    return nc


def _get_kernel(C, TS):
    key = (C, TS)
    if key not in _BUILD_CACHE:
        _BUILD_CACHE[key] = _build(C, TS)
    return _BUILD_CACHE[key]


def _pm(a, po):
    """[N, po*128] -> partition-major [128, po, N] contiguous."""
    n = a.shape[0]
    return np.ascontiguousarray(
        a.T.reshape(po, 128, n).transpose(1, 0, 2))


def kernel(x, gate_w, gate_b, w1, w2, w3, sw1, sw2, sw3):
    bf16 = ml_dtypes.bfloat16
    x = np.asarray(x)
    gate_w = np.asarray(gate_w, dtype=np.float32)
    gate_b = np.asarray(gate_b, dtype=np.float32)
    w1 = np.asarray(w1)
    w2 = np.asarray(w2)
    w3 = np.asarray(w3)
    sw1 = np.asarray(sw1)
    sw2 = np.asarray(sw2)
    sw3 = np.asarray(sw3)

    B, S, Dx = x.shape
    assert Dx == D
    T = B * S
    TS = T // N_CORES
    xt = x.reshape(T, D)

    # ---- gate (fp32, mirrors reference: sqrt(softplus), top-2 on biased) ----
    xf = xt.astype(np.float32)
    logits = xf @ gate_w.T
    scores = np.sqrt(np.log1p(np.exp(-np.abs(logits)))
                     + np.maximum(logits, 0.0))
    biased = scores + gate_b
    idx = np.argsort(-biased, axis=1, kind="stable")[:, :TOPK]
    cw = np.zeros((T, E), dtype=np.float32)
    np.put_along_axis(cw, idx, np.take_along_axis(scores, idx, axis=1), axis=1)

    sel = np.zeros((T, E), dtype=bool)
    np.put_along_axis(sel, idx, True, axis=1)
    tok_lists = [np.nonzero(sel[:, e])[0] for e in range(E)]
    counts = np.array([len(t) for t in tok_lists])
    C = max(256, int(-(-counts.max() // 128) * 128))

    nc = _get_kernel(C, TS)

    # ---- per-core input prep ----
    # weight transforms: lhsT layouts, block-major so DMAs are contiguous
    def wA_layout(wm):  # [I, D] -> [IPO, 128, D]; [ib,pi,po*128+ic]
        return np.ascontiguousarray(
            wm.T.reshape(DPO, 128, IPO, 128).transpose(2, 1, 0, 3)
        ).reshape(IPO, 128, D)

    def wB_layout(wm):  # [D, I] -> [DPO, 128, I]; [db,pi,po*128+dc]
        return np.ascontiguousarray(
            wm.T.reshape(IPO, 128, DPO, 128).transpose(2, 1, 0, 3)
        ).reshape(DPO, 128, I)

    sw1t = wA_layout(sw1)
    sw3t = wA_layout(sw3)
    sw2t = wB_layout(sw2)

    in_maps = []
    for e in range(E):
        toks = tok_lists[e]
        cnt = len(toks)
        xg = np.zeros((C, D), dtype=bf16)
        xg[:cnt] = xt[toks]
        cwe = np.zeros((C,), dtype=np.float32)
        cwe[:cnt] = cw[toks, e]
        xs_slice = xt[e * TS:(e + 1) * TS]
        in_maps.append({
            "xr": _pm(xg, DPO),
            "xs": _pm(xs_slice, DPO),
            "cwb": np.ascontiguousarray(
                np.broadcast_to(cwe[None, :], (128, C))),
            "w1t": wA_layout(w1[e]),
            "w3t": wA_layout(w3[e]),
            "w2t": wB_layout(w2[e]),
            "sw1t": sw1t,
            "sw3t": sw3t,
            "sw2t": sw2t,
        })

    res = bass_utils.run_bass_kernel_spmd(
        nc, in_maps, core_ids=list(range(N_CORES)))
    global LAST_RESULT
    LAST_RESULT = res

    # ---- unshard + combine (bf16, reference addition order) ----
    y = np.zeros((T, D), dtype=bf16)
    for e in range(E):
        toks = tok_lists[e]
        cnt = len(toks)
        ye = res.results[e]["ye"]                       # [128, DPO, C]
        ye_tok = ye.transpose(2, 1, 0).reshape(C, D)    # [c, d]
        y[toks] = y[toks] + ye_tok[:cnt]
    z = np.concatenate(
        [res.results[e]["zs"].transpose(2, 1, 0).reshape(TS, D)
         for e in range(E)], axis=0)
    out = (y + z).reshape(B, S, D)
    return out.astype(x.dtype)



# revision 2
# speedup vs baseline: 1.1965x; 1.1965x over previous
"""MoE (8 routed experts, top-2, + shared expert) on 8 TRN2 NeuronCores.

Strategy: expert-parallel with load-balanced segmentation. Host computes
the gate (fp32 numpy, mirroring the reference), then packs the 8192
routed (token, expert) pairs into 16 expert-pure segments — 8 of size S1
and 8 of size S2 (sizes chosen per the actual expert counts so
S1+S2 ~= 1058 vs the 1152 max-count padding of naive expert-parallel).
Each core runs three SwiGLU jobs: a 512-token slice of the shared
expert (first: smallest x, fastest startup), one S1 segment, one S2
segment. Segment sizes are kept >= ~256 columns so the PE matmul stream
hides the 128x128 LDWEIGHTS loads.

All device tensors are pre-arranged on host into partition-major
layouts so every DMA is contiguous per partition: activations/weights
for matmul lhsT/rhs always have the contraction dim chunked as
[pi=128, po, free].
"""

import itertools
import math

import numpy as np
import ml_dtypes

import concourse.mybir as mybir
from concourse import bacc
from concourse.tile import TileContext
from concourse import bass_utils

BF16 = mybir.dt.bfloat16
F32 = mybir.dt.float32

D = 2048          # model dim
I = 1408          # expert inter dim
E = 8             # routed experts
TOPK = 2
N_CORES = 8
DPO = D // 128    # 16 chunks of the model dim
IPO = I // 128    # 11 chunks of the inter dim

_BUILD_CACHE = {}


def _c_blocks(C):
    """Split C columns into blocks <= 512 (PSUM bank limit), greedy-max."""
    blocks = []
    off = 0
    while off < C:
        w = min(512, C - off)
        blocks.append((off, w))
        off += w
    return blocks


def _build(sizes, TS):
    """Per-core Bass kernel: shared job (TS tokens) + one routed job per
    entry in `sizes`. Same NEFF runs SPMD on all 8 cores."""
    nc = bacc.Bacc("TRN2", debug=False, enable_asserts=False,
                   num_devices=N_CORES, enable_partition_id=False)

    def din(name, shape, dt=BF16):
        return nc.dram_tensor(name, shape, dt, kind="ExternalInput").ap()

    def dout(name, shape, dt=BF16):
        return nc.dram_tensor(name, shape, dt, kind="ExternalOutput").ap()

    Silu = mybir.ActivationFunctionType.Silu

    # jobs: (name, C, scaled, paired)
    jobs = [("s", TS, False, True)]
    for j, sz in enumerate(sizes):
        jobs.append((f"r{j}", sz, True, False))

    ins = {}
    for jn, CJ, scaled, _ in jobs:
        ins[jn] = {
            "x": din(f"x_{jn}", [128, DPO, CJ]),
            "w1": din(f"w1_{jn}", [IPO, 128, D]),
            "w3": din(f"w3_{jn}", [IPO, 128, D]),
            "w2": din(f"w2_{jn}", [DPO, 128, I]),
            "y": dout(f"y_{jn}", [128, DPO, CJ]),
        }
        if scaled:
            ins[jn]["cw"] = din(f"cw_{jn}", [128, CJ], F32)

    with TileContext(nc) as tc:
        with tc.tile_pool(name="main", bufs=1) as pool, \
             tc.tile_pool(name="psum", bufs=1, space="PSUM") as pp:

            def w13_tile(jn, i, wn):
                return pool.tile([128, DPO, 128], BF16, tag="w13", bufs=8,
                                 name=f"{wn}_{jn}_{i}")

            def w13_src(jn, wn, i):
                return ins[jn][wn][i].rearrange("p (a b) -> p a b", a=DPO)

            def w2_tile(jn, do):
                return pool.tile([128, IPO, 128], BF16, tag="w2", bufs=5,
                                 name=f"w2_{jn}_{do}")

            x_sb = {}
            H_sb = {}
            cw_sb = {}
            for jn, CJ, scaled, _ in jobs:
                x_sb[jn] = pool.tile([128, DPO, CJ], BF16, tag=f"x_{jn}",
                                     bufs=1, name=f"x_{jn}")
                H_sb[jn] = pool.tile([128, IPO, CJ], BF16, tag=f"H_{jn}",
                                     bufs=1, name=f"H_{jn}")
                if scaled:
                    cw_sb[jn] = pool.tile([128, CJ], F32, tag=f"cw_{jn}",
                                          bufs=1, name=f"cw_{jn}")

            # ---- startup DMAs: shared job's first i-pair weights + its x,
            # interleaved fine-grained so the first matmul starts ~1-2us in
            # and the stream never starves.
            pre_w = {}
            for i in (0, 1):
                for wn in ("w1", "w3"):
                    pre_w[("s", i, wn)] = w13_tile("s", i, wn)
            xs = x_sb["s"]
            xd = ins["s"]["x"]
            # d-chunks of 4 for each of the 4 weight tiles, interleaved
            # with x d-slices (col-split for ring parallelism on d0)
            for c in range(0, DPO, 4):
                for i in (0, 1):
                    for wn in ("w1", "w3"):
                        nc.sync.dma_start(
                            pre_w[("s", i, wn)][:, c:c + 4, :],
                            w13_src("s", wn, i)[:, c:c + 4, :])
                if c == 0:
                    q = TS // 4
                    for b in range(4):
                        nc.sync.dma_start(xs[:, 0, b * q:(b + 1) * q],
                                          xd[:, 0, b * q:(b + 1) * q])
                    for dsl in range(1, 4):
                        nc.sync.dma_start(xs[:, dsl, :], xd[:, dsl, :])
                else:
                    for dsl in range(c, c + 4):
                        if dsl >= 4:
                            nc.sync.dma_start(xs[:, dsl, :], xd[:, dsl, :])

            def phase_a(jn, CJ, scaled, paired, pre=None):
                cbs = _c_blocks(CJ)
                H = H_sb[jn]
                x_t = x_sb[jn]
                istep = 2 if paired else 1
                for p0 in range(0, IPO, istep):
                    ii = [i for i in range(p0, min(p0 + istep, IPO))]
                    wts = {}
                    for i in ii:
                        for wn in ("w1", "w3"):
                            if pre is not None and (jn, i, wn) in pre:
                                wts[(i, wn)] = pre[(jn, i, wn)]
                            else:
                                t = w13_tile(jn, i, wn)
                                nc.sync.dma_start(t[:], w13_src(jn, wn, i))
                                wts[(i, wn)] = t
                    if jn == "s" and p0 == 2:
                        # shared stream is rolling: enqueue the routed
                        # jobs' x and cw now (needed ~100us later)
                        for jn2, CJ2, scaled2, _ in jobs[1:]:
                            for dsl in range(DPO):
                                nc.sync.dma_start(
                                    x_sb[jn2][:, dsl, :],
                                    ins[jn2]["x"][:, dsl, :])
                            nc.sync.dma_start(cw_sb[jn2][:],
                                              ins[jn2]["cw"][:])
                    ps = {}
                    for i in ii:
                        for op in (1, 3):
                            for bi, (off, w) in enumerate(cbs):
                                ps[(i, op, bi)] = pp.tile(
                                    [128, w], F32, tag="ps", bufs=8,
                                    name=f"p{op}_{jn}_{i}_{bi}")
                    for d in range(DPO):
                        for i in ii:
                            for op in (1, 3):
                                w_sb = wts[(i, "w1" if op == 1 else "w3")]
                                for bi, (off, w) in enumerate(cbs):
                                    nc.tensor.matmul(
                                        ps[(i, op, bi)][:], w_sb[:, d, :],
                                        x_t[:, d, off:off + w],
                                        start=(d == 0), stop=(d == DPO - 1))
                    for i in ii:
                        for bi, (off, w) in enumerate(cbs):
                            s_t = pool.tile([128, w], F32, tag="act1",
                                            bufs=6, name=f"s_{jn}_{i}_{bi}")
                            nc.scalar.activation(s_t[:], ps[(i, 1, bi)][:],
                                                 Silu)
                            if scaled:
                                t_t = pool.tile([128, w], F32, tag="act2",
                                                bufs=6,
                                                name=f"t_{jn}_{i}_{bi}")
                                nc.vector.tensor_mul(
                                    t_t[:], ps[(i, 3, bi)][:],
                                    cw_sb[jn][:, off:off + w])
                                nc.vector.tensor_mul(H[:, i, off:off + w],
                                                     s_t[:], t_t[:])
                            else:
                                nc.vector.tensor_mul(H[:, i, off:off + w],
                                                     s_t[:],
                                                     ps[(i, 3, bi)][:])

            def phase_b(jn, CJ, pre_w2=None):
                cbs = _c_blocks(CJ)
                H = H_sb[jn]
                for do in range(DPO):
                    if pre_w2 is not None and do == 0:
                        w2_sb = pre_w2
                    else:
                        w2_sb = w2_tile(jn, do)
                        nc.sync.dma_start(
                            w2_sb[:],
                            ins[jn]["w2"][do].rearrange("p (a b) -> p a b",
                                                        a=IPO))
                    pys = []
                    for bi, (off, w) in enumerate(cbs):
                        pys.append(pp.tile([128, w], F32, tag="ps", bufs=8,
                                           name=f"py_{jn}_{do}_{bi}"))
                    for i in range(IPO):
                        for bi, (off, w) in enumerate(cbs):
                            nc.tensor.matmul(
                                pys[bi][:], w2_sb[:, i, :],
                                H[:, i, off:off + w],
                                start=(i == 0), stop=(i == IPO - 1))
                    for bi, (off, w) in enumerate(cbs):
                        y_t = pool.tile([128, w], BF16, tag="yo", bufs=8,
                                        name=f"y_{jn}_{do}_{bi}")
                        nc.vector.tensor_copy(y_t[:], pys[bi][:])
                        nc.sync.dma_start(
                            ins[jn]["y"][:, do, off:off + w], y_t[:])

            # ---- job sequence with cross-job weight prefetch ----
            njobs = len(jobs)
            for jidx, (jn, CJ, scaled, paired) in enumerate(jobs):
                pre = pre_w if jidx == 0 else pre_next
                phase_a(jn, CJ, scaled, paired, pre=pre)
                # prefetch next job's first weight pair before our phase B
                pre_next = {}
                if jidx + 1 < njobs:
                    jn2 = jobs[jidx + 1][0]
                    for i in (0,):
                        for wn in ("w1", "w3"):
                            t = w13_tile(jn2, i, wn)
                            nc.sync.dma_start(t[:], w13_src(jn2, wn, i))
                            pre_next[(jn2, i, wn)] = t
                # prefetch our w2[do=0]
                w2_first = w2_tile(jn, 0)
                nc.sync.dma_start(
                    w2_first[:],
                    ins[jn]["w2"][0].rearrange("p (a b) -> p a b", a=IPO))
                phase_b(jn, CJ, pre_w2=w2_first)

    nc.finalize()
    return nc


def _get_kernel(sizes, TS):
    key = (tuple(sizes), TS)
    if key not in _BUILD_CACHE:
        _BUILD_CACHE[key] = _build(tuple(sizes), TS)
    return _BUILD_CACHE[key]


# ---------------- host-side planning ----------------

def _plan_sizes(counts):
    """Choose (s1, s2) segment sizes and per-expert allocation
    (k1_e, k2_e) minimizing modeled PE stream time, with every segment
    >= 256 columns so matmul streaming hides LDWEIGHTS."""
    counts = list(counts)
    ne = len(counts)
    LDW = 107.0

    def chunk_ns(C):
        if C <= 0:
            return 0.0
        nblk = -(-C // 512)
        return max(LDW, C / 2.4 + 2.5 * nblk)

    def feas_s2(resid, s2):
        return sum(-(-r // s2) for r in resid if r > 0) <= ne

    cands = sorted({-(-n // j) for n in counts for j in (1, 2, 3)} |
                   {max(counts)})
    best = None
    for s1 in cands:
        if s1 < 256:
            continue
        caps = [min(3, -(-n // s1)) for n in counts]
        for k1 in itertools.product(*[range(c + 1) for c in caps]):
            if sum(k1) > ne:
                continue
            resid = [max(0, n - k * s1) for n, k in zip(counts, k1)]
            if all(r == 0 for r in resid):
                if sum(k1) <= ne and None is not None:
                    pass
                # single-class solution (second class unused → skip;
                # handled by the 1-class candidate below)
                continue
            lo, hi = 256, max(counts)
            if not feas_s2(resid, hi):
                continue
            while lo < hi:
                mid = (lo + hi) // 2
                if feas_s2(resid, mid):
                    hi = mid
                else:
                    lo = mid + 1
            t = 528 * (chunk_ns(s1) + chunk_ns(lo))
            if best is None or t < best[0]:
                k2 = [-(-r // lo) if r > 0 else 0 for r in resid]
                best = (t, (s1, lo), list(k1), k2)
    # 1-class fallback: every expert one segment of max(counts)
    t1 = 528 * chunk_ns(max(counts))
    if best is None or t1 < best[0]:
        best = (t1, (max(counts),), [1] * ne, [0] * ne)
    _, sizes, k1, k2 = best
    return sizes, k1, k2


def _pm(a, po):
    """[N, po*128] -> partition-major [128, po, N] contiguous."""
    n = a.shape[0]
    return np.ascontiguousarray(
        a.T.reshape(po, 128, n).transpose(1, 0, 2))


def _wA_layout(wm):  # [I, D] -> [IPO, 128, D]; [ib,pi,po*128+ic]
    return np.ascontiguousarray(
        wm.T.reshape(DPO, 128, IPO, 128).transpose(2, 1, 0, 3)
    ).reshape(IPO, 128, D)


def _wB_layout(wm):  # [D, I] -> [DPO, 128, I]; [db,pi,po*128+dc]
    return np.ascontiguousarray(
        wm.T.reshape(IPO, 128, DPO, 128).transpose(2, 1, 0, 3)
    ).reshape(DPO, 128, I)


def kernel(x, gate_w, gate_b, w1, w2, w3, sw1, sw2, sw3):
    bf16 = ml_dtypes.bfloat16
    x = np.asarray(x)
    gate_w = np.asarray(gate_w, dtype=np.float32)
    gate_b = np.asarray(gate_b, dtype=np.float32)
    w1 = np.asarray(w1)
    w2 = np.asarray(w2)
    w3 = np.asarray(w3)
    sw1 = np.asarray(sw1)
    sw2 = np.asarray(sw2)
    sw3 = np.asarray(sw3)

    B, S, Dx = x.shape
    assert Dx == D
    T = B * S
    TS = T // N_CORES
    xt = x.reshape(T, D)

    # ---- gate (fp32, mirrors reference: sqrt(softplus), top-2 on biased) ----
    xf = xt.astype(np.float32)
    logits = xf @ gate_w.T
    scores = np.sqrt(np.log1p(np.exp(-np.abs(logits)))
                     + np.maximum(logits, 0.0))
    biased = scores + gate_b
    idx = np.argsort(-biased, axis=1, kind="stable")[:, :TOPK]
    cw = np.zeros((T, E), dtype=np.float32)
    np.put_along_axis(cw, idx, np.take_along_axis(scores, idx, axis=1), axis=1)

    sel = np.zeros((T, E), dtype=bool)
    np.put_along_axis(sel, idx, True, axis=1)
    tok_lists = [np.nonzero(sel[:, e])[0] for e in range(E)]
    counts = [len(t) for t in tok_lists]

    sizes, k1, k2 = _plan_sizes(counts)

    # build per-class piece lists: (expert, token_idx_array)
    nclass = len(sizes)
    pieces = [[] for _ in range(nclass)]
    for e in range(E):
        toks = tok_lists[e]
        pos = 0
        alloc = [(0, k1[e])] + ([(1, k2[e])] if nclass > 1 else [])
        for cls, k in alloc:
            for _ in range(k):
                if pos >= len(toks):
                    break
                take = min(sizes[cls], len(toks) - pos)
                pieces[cls].append((e, toks[pos:pos + take]))
                pos += take
        assert pos == len(toks), f"expert {e} unplaced tokens"
    for cls in range(nclass):
        assert len(pieces[cls]) <= N_CORES, \
            f"class {cls} needs {len(pieces[cls])} > {N_CORES} segments"
        while len(pieces[cls]) < N_CORES:
            pieces[cls].append((0, np.array([], dtype=np.int64)))

    nc = _get_kernel(sizes, TS)

    # weight layout transforms, cached per expert
    wa_cache, wb_cache = {}, {}

    def get_w(e):
        if e not in wa_cache:
            wa_cache[e] = (_wA_layout(w1[e]), _wA_layout(w3[e]))
            wb_cache[e] = _wB_layout(w2[e])
        return wa_cache[e][0], wa_cache[e][1], wb_cache[e]

    sw1t = _wA_layout(sw1)
    sw3t = _wA_layout(sw3)
    sw2t = _wB_layout(sw2)

    in_maps = []
    for c in range(N_CORES):
        m = {
            "x_s": _pm(xt[c * TS:(c + 1) * TS], DPO),
            "w1_s": sw1t, "w3_s": sw3t, "w2_s": sw2t,
        }
        for cls in range(nclass):
            e, toks = pieces[cls][c]
            CJ = sizes[cls]
            xg = np.zeros((CJ, D), dtype=bf16)
            cwe = np.zeros((CJ,), dtype=np.float32)
            cnt = len(toks)
            if cnt:
                xg[:cnt] = xt[toks]
                cwe[:cnt] = cw[toks, e]
            w1t, w3t, w2t = get_w(e)
            jn = f"r{cls}"
            m[f"x_{jn}"] = _pm(xg, DPO)
            m[f"cw_{jn}"] = np.ascontiguousarray(
                np.broadcast_to(cwe[None, :], (128, CJ)))
            m[f"w1_{jn}"] = w1t
            m[f"w3_{jn}"] = w3t
            m[f"w2_{jn}"] = w2t
        in_maps.append(m)

    res = bass_utils.run_bass_kernel_spmd(
        nc, in_maps, core_ids=list(range(N_CORES)))
    global LAST_RESULT
    LAST_RESULT = res

    # ---- unshard + combine (bf16, reference expert order) ----
    y = np.zeros((T, D), dtype=bf16)
    for e in range(E):
        for cls in range(nclass):
            for c in range(N_CORES):
                pe, toks = pieces[cls][c]
                if pe != e or len(toks) == 0:
                    continue
                CJ = sizes[cls]
                ye = res.results[c][f"y_r{cls}"]          # [128, DPO, CJ]
                ye_tok = ye.transpose(2, 1, 0).reshape(CJ, D)
                y[toks] = y[toks] + ye_tok[:len(toks)]
    z = np.concatenate(
        [res.results[c]["y_s"].transpose(2, 1, 0).reshape(TS, D)
         for c in range(N_CORES)], axis=0)
    out = (y + z).reshape(B, S, D)
    return out.astype(x.dtype)
